# revision 28
# baseline (speedup 1.0000x reference)
"""CondNet kernel for Trainium2 (8 NeuronCores, model-parallel mid layers).

v4 over v3 (v3: 480us - 16 fine-grained AllGathers serialized on the
CC core at ~13.5us each and the CC core does not engage until ~70us
after NEFF start, so the PE starved):
  - Work WITH the ~70us CC-boot constraint instead of against it:
    every core redundantly computes L1 for output-chunks 0..3 of ALL
    cores (32 replicated strips, pure data-parallel SPMD, weights fed
    via a replicated w_in_rep tensor) plus its own otiles 4..7.  The
    first half of each mid layer's k-range is then LOCAL: no gather
    needed.  The replicated strips fill the PE during the CC-boot
    window that was previously dead time.
  - ONE h1 AllGather (own otiles 4..7, 0.5 MB contribution) rides the
    CC boot: lands ~104us, first needed ~130us.
  - L2: kt-major over local kts 0..31, then otile-major tail over
    gathered kts 32..63; each otile-pair evicts early and fires one of
    4 h2 quarter-AllGathers, staggered so all land before L3's need.
  - L3: pure kt-major (chunks arrive early); h3 stays in SBUF.
  - L4: per-core k-partials, host sum (as v2/v3).
"""

import os
import numpy as np
import ml_dtypes

import concourse.bass as bass
import concourse.tile as tile
from concourse import bacc, mybir
from concourse import bass_utils

BF16 = ml_dtypes.bfloat16

B, NUM_IN, NUM_MID, NUM_OUT, FAN_IN, N_COND = 512, 1024, 8192, 1000, 64, 2
NCORES = 8
OSLAB = NUM_MID // NCORES      # 1024 outputs per core for mid layers
RG = [list(range(NCORES))]     # one replica group: all 8 cores
NREP = 4                       # chunks 0..NREP-1 computed locally on every core

_cache = {}
LAST_RESULT = None  # BassKernelResults of the most recent run (for test harness)


def _build_nc():
    """Build + compile the Bass program (same SPMD program for all 8 cores)."""
    nc = bacc.Bacc("TRN2", target_bir_lowering=False, debug=False, num_devices=NCORES)
    f32, bf16 = mybir.dt.float32, mybir.dt.bfloat16

    NOWN = 8 - NREP            # own otiles NREP..7 contributed to the AG
    NSTRIP = NOWN + 8 * NREP   # L1 strips computed per core

    # ---- DRAM I/O (host-pretransposed; all DMAs contiguous) ----
    xT_d = nc.dram_tensor("xT", [128, 8, B], bf16, kind="ExternalInput").ap()
    w_own_d = nc.dram_tensor("w_own", [NOWN, 128, 8, 128], bf16,
                             kind="ExternalInput").ap()
    w_rep_d = nc.dram_tensor("w_rep", [8 * NREP, 128, 8, 128], bf16,
                             kind="ExternalInput").ap()
    b_in_d = nc.dram_tensor("b_in", [128, NSTRIP], f32, kind="ExternalInput").ap()
    s1_d = nc.dram_tensor("s1", [64, 128, 1024], bf16, kind="ExternalInput").ap()
    b1_d = nc.dram_tensor("b1", [128, 8], f32, kind="ExternalInput").ap()
    s2_d = nc.dram_tensor("s2", [64, 128, 1024], bf16, kind="ExternalInput").ap()
    b2_d = nc.dram_tensor("b2", [128, 8], f32, kind="ExternalInput").ap()
    w_out_d = nc.dram_tensor("w_out", [128, 8, 1024], bf16, kind="ExternalInput").ap()
    out_d = nc.dram_tensor("out", [8, 128, B], f32, kind="ExternalOutput").ap()

    with tile.TileContext(nc) as tc:
        with (
            tc.tile_pool(name="const", bufs=1) as cpool,
            tc.tile_pool(name="wstrip", bufs=6) as wpool,
            tc.tile_pool(name="sstream", bufs=38) as spool,
            tc.tile_pool(name="hstream", bufs=44) as hpool,
            tc.tile_pool(name="acts", bufs=6) as apool,
            tc.tile_pool(name="psmm", bufs=1, space="PSUM") as pmm,
            tc.tile_pool(name="dram", bufs=1, space="DRAM") as dpool,
        ):
            # ---- persistent SBUF tensors ----
            xT = cpool.tile([128, 8, B], bf16)
            nc.sync.dma_start(out=xT[:, 0:4, :], in_=xT_d[:, 0:4, :])
            nc.scalar.dma_start(out=xT[:, 4:8, :], in_=xT_d[:, 4:8, :])
            b_in = cpool.tile([128, NSTRIP], f32)
            b1 = cpool.tile([128, 8], f32)
            b2 = cpool.tile([128, 8], f32)
            nc.scalar.dma_start(out=b_in[:], in_=b_in_d)
            nc.scalar.dma_start(out=b1[:], in_=b1_d)
            nc.scalar.dma_start(out=b2[:], in_=b2_d)

            # DRAM bounce + gathered buffers
            h1b = dpool.tile([NOWN, 128, B], bf16, tag="h1b", name="h1b")
            h1g = dpool.tile([8 * NOWN, 128, B], bf16, tag="h1g", name="h1g",
                             addr_space="Shared")
            h2b = [dpool.tile([2, 128, B], bf16, tag=f"h2b{q}", name=f"h2b{q}")
                   for q in range(4)]
            h2g = [dpool.tile([16, 128, B], bf16, tag=f"h2g{q}", name=f"h2g{q}",
                              addr_space="Shared")
                   for q in range(4)]

            psums = [pmm.tile([128, B], f32, tag=f"mm{ot}", name=f"ps{ot}")
                     for ot in range(8)]

            # ---- L1 strips: own otiles NREP..7 first (feed the AG),
            # then 8*NREP replicated strips (local h tiles, kts 0..8*NREP-1)
            def strip(i, w_src, bias_col, out_tile, bank=None):
                wt = wpool.tile([128, 8, 128], bf16, tag="w", name="wt")
                # keep scalar free of w-issues: its in-order queue otherwise
                # interleaves [w-issue, ACTIVATE, ...] and each ACTIVATE's
                # semaphore wait blocks the w-issues queued behind it
                eng = nc.sync if i % 2 == 0 else nc.gpsimd
                eng.dma_start(out=wt[:], in_=w_src)
                ps = psums[i % 8 if bank is None else bank]
                for kt in range(8):
                    nc.tensor.matmul(ps[:], wt[:, kt, :], xT[:, kt, :],
                                     start=(kt == 0), stop=(kt == 7))
                nc.scalar.activation(
                    out_tile[:], ps[:], mybir.ActivationFunctionType.Relu,
                    bias=b_in[:, bias_col:bias_col + 1])

            for j in range(NOWN):
                act = apool.tile([128, B], bf16, tag="act", name="act")
                strip(j, w_own_d[j], j, act)
                nc.gpsimd.dma_start(out=h1b[j], in_=act[:])
            nc.gpsimd.collective_compute(
                "AllGather", mybir.AluOpType.bypass, replica_groups=RG,
                ins=[h1b.opt()], outs=[h1g.opt()])

            # ---- ~13us PE pause on the vector engine (gpsimd now carries
            # w-odd issues): anchor on own-strip-3's psum so the scheduler
            # cannot hoist the chain; gates on all 8 psum banks WAR-block
            # the replicated strips' start=True until the chain completes.
            sc = cpool.tile([128, 1024], f32)
            sc2 = cpool.tile([128, B], f32)
            nc.vector.tensor_scalar(sc[:, 0:B], sc[:, 0:B], psums[3][:, 0:1],
                                    None, mybir.AluOpType.add)
            for _ in range(20):
                nc.vector.memset(sc[:], 0.0)
            for j in range(8):
                nc.vector.tensor_scalar(sc2[:], sc[:, 0:B], psums[j][:, 0:1],
                                        None, mybir.AluOpType.add)

            local_h = []
            for s in range(8 * NREP):
                ht = hpool.tile([128, B], bf16, tag="h", name="ht")
                strip(NOWN + s, w_rep_d[s], NOWN + s, ht)
                local_h.append(ht)

            # ---- S/h streaming helpers (single-ktile tiles: 2 KB partition
            # stride; 4 KB-stride pair tiles measurably slow the PE) ----
            def s_tile_fn(s_d, sts):
                def get(kt):
                    if kt not in sts:
                        se = nc.sync if kt % 2 == 0 else nc.scalar
                        st = spool.tile([128, 1024], bf16, tag="s", name="st")
                        se.dma_start(out=st[:], in_=s_d[kt])
                        sts[kt] = st
                    return sts[kt]
                return get

            # ---- L2: kt-major local head, otile-major gathered tail ----
            s_l2 = s_tile_fn(s1_d, {})
            for kt in range(8 * NREP):
                st, ht = s_l2(kt), local_h[kt]
                for ot in range(8):
                    nc.tensor.matmul(psums[ot][:],
                                     st[:, ot * 128:(ot + 1) * 128], ht[:],
                                     start=(kt == 0), stop=False)
            tail_s = [s_l2(kt) for kt in range(8 * NREP, 64)]
            tail_h = []
            for kt in range(8 * NREP, 64):
                c, r = kt // 8, kt % 8
                ht = hpool.tile([128, B], bf16, tag="h", name="ht")
                he = nc.sync if kt % 2 == 0 else nc.scalar
                he.dma_start(out=ht[:], in_=h1g[r * NOWN + (c - NREP)])
                tail_h.append(ht)
            h2acts = []
            for ot in range(8):
                for i in range(64 - 8 * NREP):
                    nc.tensor.matmul(psums[ot][:],
                                     tail_s[i][:, ot * 128:(ot + 1) * 128],
                                     tail_h[i][:],
                                     start=False, stop=(i == 63 - 8 * NREP))
                act = apool.tile([128, B], bf16, tag="act", name="act")
                nc.scalar.activation(
                    act[:], psums[ot][:], mybir.ActivationFunctionType.Relu,
                    bias=b1[:, ot:ot + 1])
                h2acts.append(act)
                nc.gpsimd.dma_start(out=h2b[ot // 2][ot % 2], in_=act[:])
                if ot % 2 == 1:
                    q = ot // 2
                    nc.gpsimd.collective_compute(
                        "AllGather", mybir.AluOpType.bypass, replica_groups=RG,
                        ins=[h2b[q].opt()], outs=[h2g[q].opt()])

            # ---- L3: pure kt-major over gathered h2; h3 stays in SBUF ----
            s_l3 = s_tile_fn(s2_d, {})
            for kt in range(64):
                c, r = kt // 8, kt % 8
                ht = hpool.tile([128, B], bf16, tag="h", name="ht")
                he = nc.scalar if kt % 2 == 0 else nc.sync
                he.dma_start(out=ht[:], in_=h2g[c // 2][r * 2 + (c % 2)])
                st = s_l3(kt)
                for ot in range(8):
                    nc.tensor.matmul(psums[ot][:],
                                     st[:, ot * 128:(ot + 1) * 128], ht[:],
                                     start=(kt == 0), stop=(kt == 63))
            h3 = []
            for ot in range(8):
                act = cpool.tile([128, B], bf16, tag=f"h3k{ot}", name=f"h3k{ot}")
                nc.scalar.activation(
                    act[:], psums[ot][:], mybir.ActivationFunctionType.Relu,
                    bias=b2[:, ot:ot + 1])
                h3.append(act)

            # w_out load deferred so its 2 MB doesn't delay S2
            w_out = cpool.tile([128, 8, 1024], bf16)
            nc.sync.dma_start(out=w_out[:, 0:4, :], in_=w_out_d[:, 0:4, :])
            nc.scalar.dma_start(out=w_out[:, 4:8, :], in_=w_out_d[:, 4:8, :])

            # ---- L4: jt-major k-partials, evict + DMA-out per jt ----
            for jt in range(8):
                for kt in range(8):
                    nc.tensor.matmul(psums[jt][:],
                                     w_out[:, kt, jt * 128:(jt + 1) * 128],
                                     h3[kt][:],
                                     start=(kt == 0), stop=(kt == 7))
                osb = apool.tile([128, B], f32, tag="out", name="osb")
                nc.vector.tensor_copy(osb[:], psums[jt][:])
                nc.gpsimd.dma_start(out=out_d[jt], in_=osb[:])

    nc.compile()
    return nc


def _perm():
    """k-order of the gathered activations: 8 chunks, core-major inside."""
    return np.concatenate(
        [np.arange(r * OSLAB + c * 128, r * OSLAB + (c + 1) * 128)
         for c in range(8) for r in range(NCORES)])


def _prep_inputs(x, W_in, b_in, W_mid, b_mid, W_out, b_out, indx_seqs):
    """Host-side compile-time transforms of inputs (per-core slabs)."""
    idx = np.asarray(indx_seqs).astype(np.int64)
    perm = _perm()
    NOWN = 8 - NREP

    def build_S(Wm):
        # S[k, o] = sum_f Wm[o, f] * [idx[o, f] == k], k rows permuted
        S = np.zeros((NUM_MID, NUM_MID), np.float32)
        cols = np.repeat(np.arange(NUM_MID), FAN_IN)
        np.add.at(S, (idx.reshape(-1), cols), np.asarray(Wm, np.float32).reshape(-1))
        return S[perm].reshape(64, 128, NUM_MID).astype(BF16)

    s1_t = build_S(W_mid[0])
    s2_t = build_S(W_mid[1])

    x = np.asarray(x, np.float32)
    xT = np.ascontiguousarray(x.T.reshape(8, 128, B).transpose(1, 0, 2)).astype(BF16)
    w_in_t = np.asarray(W_in, np.float32).T  # [1024, 8192]
    # [kt, p, g_otile, j] view of W_in^T
    w4 = w_in_t.reshape(8, 128, 64, 128)
    b_in_f = np.asarray(b_in, np.float32)
    woT = np.asarray(W_out, np.float32).T    # [8192, 1000]

    # replicated strips: order s = c*8 + r' -> global otile r'*8 + c
    g_order = [r * 8 + c for c in range(NREP) for r in range(NCORES)]
    w_rep = np.ascontiguousarray(
        w4[:, :, g_order, :].transpose(2, 1, 0, 3)).astype(BF16)  # [8*NREP,128,8,128]

    def bias_col(g):  # bias column [128] for global otile g
        return b_in_f[g * 128:(g + 1) * 128]

    b_rep = np.stack([bias_col(g) for g in g_order], axis=1)  # [128, 8*NREP]

    def bias_slab(b, c):
        return np.ascontiguousarray(
            np.asarray(b, np.float32)[c * OSLAB:(c + 1) * OSLAB].reshape(8, 128).T)

    in_maps = []
    for c in range(NCORES):
        sl = slice(c * OSLAB, (c + 1) * OSLAB)
        own_g = [c * 8 + ot for ot in range(NREP, 8)]
        w_own = np.ascontiguousarray(
            w4[:, :, own_g, :].transpose(2, 1, 0, 3)).astype(BF16)
        b_own = np.stack([bias_col(g) for g in own_g], axis=1)   # [128, NOWN]
        b_in_slab = np.ascontiguousarray(
            np.concatenate([b_own, b_rep], axis=1))              # [128, NSTRIP]
        wo = np.zeros((OSLAB, 1024), np.float32)
        wo[:, :NUM_OUT] = woT[sl]
        wo_t = wo.reshape(8, 128, 1024).transpose(1, 0, 2)
        in_maps.append({
            "xT": xT,
            "w_own": w_own,
            "w_rep": w_rep,
            "b_in": b_in_slab,
            "s1": np.ascontiguousarray(s1_t[:, :, sl]),
            "b1": bias_slab(b_mid[0], c),
            "s2": np.ascontiguousarray(s2_t[:, :, sl]),
            "b2": bias_slab(b_mid[1], c),
            "w_out": np.ascontiguousarray(wo_t).astype(BF16),
        })
    return in_maps, np.asarray(b_out, np.float32)


def kernel(x, W_in, b_in, W_mid, b_mid, W_out, b_out, indx_seqs):
    global LAST_RESULT
    if "nc" not in _cache:
        _cache["nc"] = _build_nc()
    nc = _cache["nc"]

    in_maps, b_out_f = _prep_inputs(x, W_in, b_in, W_mid, b_mid, W_out, b_out,
                                    indx_seqs)

    res = bass_utils.run_bass_kernel_spmd(
        nc, in_maps, core_ids=list(range(NCORES)),
        trace=bool(int(os.environ.get("KERNEL_TRACE", "0"))),
    )
    LAST_RESULT = res

    acc = np.zeros((1024, B), np.float64)
    for r in res.results:
        acc += r["out"].reshape(1024, B)
    out = acc[:NUM_OUT].T + b_out_f[None, :]
    return np.ascontiguousarray(out).astype(np.float32)
